# revision 32
# baseline (speedup 1.0000x reference)
"""Trainium2 Bass kernel for nn_AttentionBlock (B=16, C=512, H=W=32, 4 heads).

Data-parallel over batch across 8 NeuronCores (2 images/core). All GEMMs run
in fp8 (e4m3) with perf_mode=DoubleRow: both operands are laid out as
[K<=128 partitions, 2, free] so each matmul contracts 2*K rows at 0.5
PE-cycles per output row. Per image (x[b] is (C, N), N = H*W = 1024 tokens):

  q,k   = wqk^T @ x          feature-major, stored [64p, 2, N] per head so the
                             d=128 head dim is itself a DoubleRow pair (p,t)
  v     = x^T @ wv           token-major [j, C]
  sT    = kT^T @ qT          (j, i) layout, one DR matmul per 128-token j-tile
  e     = exp(sT*scale - 5)  ACT, fp8 out; constant shift keeps e in [0, ~15]
                             (softmax is shift-invariant; D uses the same e)
  D     = ones^T @ e         column sums via fp8 DR matmul, fp32 PSUM
  res   = (v^T @ e) / D      PV via DR matmul; DVE reciprocal + multiply
  y     = wout^T @ res + b_eff + x

Bias handling: q/k biases are added at PSUM->SBUF evacuation (per-partition
scalars); the v bias commutes through the softmax average and folds into
b_eff = b_out + b_v @ w_out on the host.
"""

import numpy as np
import ml_dtypes

import concourse.bass as bass
import concourse.mybir as mybir
import concourse.tile as tile
from concourse import bacc
from concourse.bass_utils import run_bass_kernel_spmd

dt = mybir.dt
F8NP = ml_dtypes.float8_e4m3
DR = mybir.MatmulPerfMode.DoubleRow

N_CORES = 8
B = 16
C = 512
HEADS = 4
DK = C // HEADS          # 128
N = 1024                 # H*W tokens
SCALE = float(DK) ** -0.5
SHIFT = -5.0             # exp(s*scale + SHIFT) <= ~15 fits e4m3 (max 240)
BPC = B // N_CORES       # batches per core = 2
CT = C // 128            # 4 contraction tiles over channels
CTP = CT // 2            # 2 DoubleRow contraction pairs
NB = N // 512            # 2 moving-dim blocks of 512 tokens
JT = N // 128            # 8 key-token tiles
JP = JT // 2             # 4 DoubleRow j-tile pairs

LAST_RESULTS = None  # BassKernelResults of the most recent run (for test.py)


def build_program():
    nc = bacc.Bacc("TRN2", target_bir_lowering=False, debug=False,
                   num_devices=N_CORES)

    x = nc.dram_tensor("x", [BPC, C, N], dt.float32, kind="ExternalInput").ap()
    x8 = nc.dram_tensor("x8", [BPC, C, N], dt.float8e4, kind="ExternalInput").ap()
    # contraction index c = ctp*256 + ko*128 + p for all three weights
    wqk = nc.dram_tensor("wqk", [128, CTP, 2, 8, 128], dt.float8e4,
                         kind="ExternalInput").ap()
    wv = nc.dram_tensor("wv", [128, CTP, 2, C], dt.float8e4,
                        kind="ExternalInput").ap()
    wout = nc.dram_tensor("wout", [128, CTP, 2, CT, 128], dt.float8e4,
                          kind="ExternalInput").ap()
    ones = nc.dram_tensor("ones", [128, 2, 128], dt.float8e4,
                          kind="ExternalInput").ap()
    bqk = nc.dram_tensor("bqk", [128, 8], dt.float32, kind="ExternalInput").ap()
    beff = nc.dram_tensor("beff", [128, CT], dt.float32,
                          kind="ExternalInput").ap()
    y = nc.dram_tensor("y", [BPC, C, N], dt.float32, kind="ExternalOutput").ap()

    with tile.TileContext(nc) as tc:
        with (
            tc.tile_pool(name="weights", bufs=1) as wpool,
            tc.tile_pool(name="xin", bufs=2) as xpool,
            tc.tile_pool(name="x8", bufs=2) as x8pool,
            tc.tile_pool(name="qk", bufs=2) as qkpool,
            tc.tile_pool(name="vbuf", bufs=2) as vpool,
            tc.tile_pool(name="ebuf", bufs=4) as epool,
            tc.tile_pool(name="dbuf", bufs=2) as dpool,
            tc.tile_pool(name="res", bufs=2) as rpool,
            tc.tile_pool(name="yout", bufs=4) as ypool,
            tc.tile_pool(name="ps_s", bufs=2, space="PSUM") as ps_s,
            tc.tile_pool(name="ps_p", bufs=2, space="PSUM") as ps_p,
            tc.tile_pool(name="ps_d", bufs=1, space="PSUM") as ps_d,
            tc.tile_pool(name="ps_r", bufs=1, space="PSUM") as ps_r,
        ):
            # ---- loads: x8 (fp8, host-cast) feeds the projections and is
            # on the critical path; fp32 x is residual-only and loads late.
            xT_sbs, x8_sbs = [], []
            for b in range(BPC):
                xT_sb = xpool.tile([128, CT, NB, 512], dt.float32)
                x8_sb = x8pool.tile([128, CT, NB, 512], dt.float8e4)
                xT_sbs.append(xT_sb)
                x8_sbs.append(x8_sb)

            shift_sb = wpool.tile([128, 1], dt.float32)
            nc.any.memset(shift_sb, SHIFT)
            # pre-warm the ACT exp table so the first real exp doesn't pay
            # the ~1.3us table load on the critical path
            warm_sb = wpool.tile([128, 1], dt.float32)
            nc.scalar.activation(warm_sb, shift_sb,
                                 mybir.ActivationFunctionType.Exp, scale=0.0)
            x8r = [x8[b].rearrange("(ct p) (nb n) -> p ct nb n", p=128, n=512)
                   for b in range(BPC)]
            nc.sync.dma_start(out=x8_sbs[0][:, :, 0, :], in_=x8r[0][:, :, 0, :])
            # wqk/bqk chunk axis is host-reordered to (0,1,4,5,2,3,6,7) so
            # the critical chunks are one contiguous early DMA slice
            wqk_sb = wpool.tile([128, CTP, 2, 8, 128], dt.float8e4)
            nc.sync.dma_start(out=wqk_sb[:, :, :, 0:4, :],
                              in_=wqk[:, :, :, 0:4, :])
            bqk_sb = wpool.tile([128, 8], dt.float32)
            nc.sync.dma_start(out=bqk_sb, in_=bqk)
            nc.sync.dma_start(out=x8_sbs[0][:, :, 1, :], in_=x8r[0][:, :, 1, :])
            nc.sync.dma_start(out=wqk_sb[:, :, :, 4:8, :],
                              in_=wqk[:, :, :, 4:8, :])
            wv_sb = wpool.tile([128, CTP, 2, C], dt.float8e4)
            nc.sync.dma_start(out=wv_sb, in_=wv)
            wout_sb = wpool.tile([128, CTP, 2, CT, 128], dt.float8e4)
            nc.sync.dma_start(out=wout_sb, in_=wout)
            ones_sb = wpool.tile([128, 2, 128], dt.float8e4)
            nc.sync.dma_start(out=ones_sb, in_=ones)
            beff_sb = wpool.tile([128, CT], dt.float32)
            nc.sync.dma_start(out=beff_sb, in_=beff)
            nc.sync.dma_start(out=x8_sbs[1], in_=x8r[1])
            xr = [x[b].rearrange("(ct p) (nb n) -> p ct nb n", p=128, n=512)
                  for b in range(BPC)]
            nc.sync.dma_start(out=xT_sbs[0], in_=xr[0])
            nc.sync.dma_start(out=xT_sbs[1], in_=xr[1])

            for b in range(BPC):
                xT_sb, x8_sb = xT_sbs[b], x8_sbs[b]
                # ---- q/k + v projections, interleaved so that heads
                # 0/1's score tiles unlock as early as possible ----
                # qk8[(p<64 ? head 2hp : head 2hp+1), qk, hp, t, nb, n];
                # head-dim index d = t*64 + (p mod 64)
                qk8 = qkpool.tile([128, 2, 2, 2, NB, 512], dt.float8e4)
                v8 = vpool.tile([128, JP, 2, C], dt.float8e4)

                CHUNK_POS = {0: 0, 1: 1, 4: 2, 5: 3, 2: 4, 3: 5, 6: 6, 7: 7}

                def qk_chunk(n_ch, nb, on_act=False):
                    qk, hp, t = n_ch >> 2, (n_ch >> 1) & 1, n_ch & 1
                    pos = CHUNK_POS[n_ch]
                    ps = ps_p.tile([128, 512], dt.float32, tag="psp")
                    for ctp in range(CTP):
                        nc.tensor.matmul(
                            ps,
                            wqk_sb[:, ctp, :, pos, :],
                            x8_sb[:, 2 * ctp:2 * ctp + 2, nb, :],
                            start=(ctp == 0), stop=(ctp == CTP - 1),
                            perf_mode=DR)
                    if on_act:
                        # ACT is idle before the first exp; identity+bias
                        # shares the exp table (no table reload)
                        nc.scalar.activation(
                            qk8[:, qk, hp, t, nb, :], ps,
                            mybir.ActivationFunctionType.Identity,
                            bias=bqk_sb[:, pos:pos + 1])
                    else:
                        nc.vector.tensor_scalar_add(
                            qk8[:, qk, hp, t, nb, :], ps,
                            bqk_sb[:, pos:pos + 1])

                def v_tile(jt):
                    ps = ps_p.tile([128, 512], dt.float32, tag="psp")
                    nbj, off = divmod(jt * 128, 512)
                    for ctp in range(CTP):
                        nc.tensor.matmul(
                            ps,
                            x8_sb[:, 2 * ctp:2 * ctp + 2, nbj, off:off + 128],
                            wv_sb[:, ctp, :, :],
                            start=(ctp == 0), stop=(ctp == CTP - 1),
                            perf_mode=DR)
                    nc.vector.tensor_copy(v8[:, jt // 2, jt % 2, :], ps)

                # ---- attention emission helpers ----
                yr = y[b].rearrange("(cot p) (nb n) -> p cot nb n",
                                    p=128, n=512)
                res8 = rpool.tile([128, CT, NB, 512], dt.float8e4)

                def scores_pair(h, ib, jps, e8):
                    base, hp = 64 * (h & 1), h >> 1
                    ps = ps_s.tile([128, 2, 512], dt.float32, tag="ps")
                    for half in range(2):
                        jt = 2 * jps + half
                        nbj, off = divmod(jt * 128, 512)
                        nc.tensor.matmul(
                            ps[:, half, :],
                            qk8[base:base + 64, 1, hp, :, nbj, off:off + 128],
                            qk8[base:base + 64, 0, hp, :, ib, :],
                            start=True, stop=True, perf_mode=DR)
                    nc.scalar.activation(
                        e8[:, 2 * jps:2 * jps + 2, :], ps,
                        mybir.ActivationFunctionType.Exp,
                        scale=SCALE, bias=shift_sb)

                def pv_norm(h, ib, e8):
                    psd = ps_d.tile([128, 512], dt.float32, tag="psd")
                    psr = ps_r.tile([128, 512], dt.float32, tag="psr")
                    for jp2 in range(JP):
                        epair = e8[:, 2 * jp2:2 * jp2 + 2, :]
                        nc.tensor.matmul(
                            psd, ones_sb, epair,
                            start=(jp2 == 0), stop=(jp2 == JP - 1),
                            perf_mode=DR)
                        nc.tensor.matmul(
                            psr, v8[:, jp2, :, h * DK:(h + 1) * DK], epair,
                            start=(jp2 == 0), stop=(jp2 == JP - 1),
                            perf_mode=DR)
                    d_sb = dpool.tile([128, 512], dt.float32)
                    nc.vector.reciprocal(d_sb, psd)
                    nc.vector.tensor_mul(res8[:, h, ib, :], psr, d_sb)

                def attn_head_pair(ha, hb, ib):
                    # interleave the two heads' score/exp streams so the ACT
                    # pipeline never waits on one head's input chain
                    e8a = epool.tile([128, JT, 512], dt.float8e4, tag="e8")
                    e8b = epool.tile([128, JT, 512], dt.float8e4, tag="e8")
                    for jps in range(JP):
                        scores_pair(ha, ib, jps, e8a)
                        scores_pair(hb, ib, jps, e8b)
                    pv_norm(ha, ib, e8a)
                    pv_norm(hb, ib, e8b)

                def out_proj(ib, ctp_order=(0, 1), split_dma=False):
                    # ctp_order: put the earlier-finishing head pair first so
                    # the accumulation can start before the last mul lands
                    for cot in range(CT):
                        pool_o = ps_d if cot % 2 == 0 else ps_r
                        ps = pool_o.tile([128, 512], dt.float32,
                                         tag="psd" if cot % 2 == 0 else "psr")
                        for k, ctp in enumerate(ctp_order):
                            nc.tensor.matmul(
                                ps,
                                wout_sb[:, ctp, :, cot, :],
                                res8[:, 2 * ctp:2 * ctp + 2, ib, :],
                                start=(k == 0), stop=(k == CTP - 1),
                                perf_mode=DR)
                        y_sb = ypool.tile([128, 512], dt.float32)
                        nc.vector.scalar_tensor_tensor(
                            y_sb, ps, beff_sb[:, cot:cot + 1],
                            xT_sb[:, cot, ib, :],
                            op0=mybir.AluOpType.add, op1=mybir.AluOpType.add)
                        # at the kernel tail ACT is idle: spread the final
                        # stores over both HWDGE queues
                        eng = nc.scalar if (split_dma and cot % 2) else nc.sync
                        eng.dma_start(out=yr[:, cot, ib, :], in_=y_sb)

                # ---- emission order: critical projections, heads 0/1,
                # remaining projections, heads 2/3, then ib=1 ----
                for n_ch, nb in ((0, 0), (1, 0), (4, 0), (5, 0), (4, 1),
                                 (5, 1)):
                    qk_chunk(n_ch, nb, on_act=(b == 0 and nb == 0
                                               and n_ch in (4, 5)))
                for n_ch, nb in ((2, 0), (3, 0), (6, 0), (7, 0), (6, 1),
                                 (7, 1)):
                    qk_chunk(n_ch, nb)
                for jt in range(JT):
                    v_tile(jt)
                for n_ch, nb in ((0, 1), (1, 1), (2, 1), (3, 1)):
                    qk_chunk(n_ch, nb)
                attn_head_pair(0, 1, 0)
                attn_head_pair(2, 3, 0)
                out_proj(0)
                attn_head_pair(2, 3, 1)
                attn_head_pair(0, 1, 1)
                out_proj(1, ctp_order=(1, 0), split_dma=(b == BPC - 1))
    nc.finalize()
    return nc


_CACHED_NC = None


def _get_program():
    global _CACHED_NC
    if _CACHED_NC is None:
        _CACHED_NC = build_program()
    return _CACHED_NC


def _pack_weights(w_proj, b_proj, w_out, b_out):
    w4 = w_proj.reshape(C, HEADS, 3, DK)
    # wqk8[c -> (ctp, ko, p), n_ch=(qk, hp, t), f]: f<64 -> head 2hp,
    # f>=64 -> head 2hp+1; d = t*64 + (f mod 64)
    arr = np.empty((C, 2, 2, 2, 128), np.float32)
    for qk in range(2):
        for hp in range(2):
            for t in range(2):
                arr[:, qk, hp, t, :64] = w4[:, 2 * hp, qk,
                                            t * 64:(t + 1) * 64]
                arr[:, qk, hp, t, 64:] = w4[:, 2 * hp + 1, qk,
                                            t * 64:(t + 1) * 64]
    wqk8 = np.ascontiguousarray(
        arr.reshape(CTP, 2, 128, 8, 128).transpose(2, 0, 1, 3, 4))
    CHUNK_ORDER = (0, 1, 4, 5, 2, 3, 6, 7)
    wqk8 = np.ascontiguousarray(wqk8[:, :, :, CHUNK_ORDER, :]).astype(F8NP)

    b4 = b_proj.reshape(HEADS, 3, DK)
    bqk = np.empty((128, 8), np.float32)
    for n_ch in range(8):
        qk, hp, t = n_ch >> 2, (n_ch >> 1) & 1, n_ch & 1
        bqk[:64, n_ch] = b4[2 * hp, qk, t * 64:(t + 1) * 64]
        bqk[64:, n_ch] = b4[2 * hp + 1, qk, t * 64:(t + 1) * 64]

    wv = w4[:, :, 2, :].reshape(C, C)
    wv8 = np.ascontiguousarray(
        wv.reshape(CTP, 2, 128, C).transpose(2, 0, 1, 3)).astype(F8NP)

    wout8 = np.ascontiguousarray(
        w_out.reshape(CTP, 2, 128, CT, 128).transpose(2, 0, 1, 3, 4)).astype(F8NP)

    # v-bias commutes through the softmax average: b_eff = b_out + b_v @ w_out
    b_eff = b_out + b4[:, 2, :].reshape(C) @ w_out
    beff = np.ascontiguousarray(b_eff.reshape(CT, 128).T)

    bqk = bqk[:, list(CHUNK_ORDER)]
    return {
        "wqk": wqk8, "wv": wv8, "wout": wout8,
        "ones": np.ones((128, 2, 128), np.float32).astype(F8NP),
        "bqk": np.ascontiguousarray(bqk), "beff": beff,
    }


def kernel(x, w_proj, b_proj, w_out, b_out):
    global LAST_RESULTS
    x = np.ascontiguousarray(np.asarray(x, dtype=np.float32)).reshape(B, C, N)
    w_proj = np.asarray(w_proj, dtype=np.float32)
    b_proj = np.asarray(b_proj, dtype=np.float32)
    w_out = np.asarray(w_out, dtype=np.float32)
    b_out = np.asarray(b_out, dtype=np.float32)

    weights = _pack_weights(w_proj, b_proj, w_out, b_out)

    x8 = x.astype(F8NP)
    nc = _get_program()
    in_maps = []
    for c in range(N_CORES):
        in_maps.append({"x": x[c * BPC:(c + 1) * BPC],
                        "x8": x8[c * BPC:(c + 1) * BPC], **weights})
    res = run_bass_kernel_spmd(nc, in_maps, list(range(N_CORES)))
    LAST_RESULTS = res
    out = np.concatenate([res.results[c]["y"] for c in range(N_CORES)], axis=0)
    return out.reshape(B, C, 32, 32)
